# revision 8
# baseline (speedup 1.0000x reference)
"""Trainium2 Bass kernel for nn_MeshNetV0 (GNN message passing), 8 cores.

Strategy (pull-mode sharding):
  - Core c owns dst nodes [c*SHARD, (c+1)*SHARD) and all their in-edges.
  - Layer-0 node features are fed pre-replicated as full-table inputs
    (tableA/B), so no layer-0 AllGather is needed.
  - Per conv layer: gpsimd dma_gather pulls per-edge source rows (bf16,
    256 B stride) into edge-major SBUF tiles [128 slots, cols, 64]; per
    128-slot column a TensorE matmul  msg[:, :64].T @ S  (S = host-built
    weighted one-hot [128, w], bf16) computes the weighted segment-sum
    into feature-major PSUM [64, node-window]; the @W runs
    post-aggregation ((A h) W) and ACT applies bias+ReLU on the drain.
  - Gathers are spread over all 4 SWDGE queues (rotating with s so the
    unequal quarter sizes average out) — desc generation runs on 4 Q7
    core pairs concurrently.
  - Node features for the next layer are re-transposed per supertile and
    staged to DRAM in two chunks (A = supertiles 0-12, B = 13-24); each
    chunk AllGathers separately so chunk A's collective overlaps the
    tail of the current layer's gathers.
  - The int16 gather indices address four table quarters: A is split in
    2x 26624 rows, B in 2x 23376 rows.
  - The 320->1024 combination + max-pool run fused into layer 3's
    supertile loop (x4 read straight from SBUF); AllReduce-max merges;
    the BN-folded FC head is replicated on every core.

The column structure (node ranges per 128-slot column) is built from
max-over-cores degree sums so the instruction stream is identical on all
8 cores; only tensor contents (gather indices, S values) differ.
"""

import math
import numpy as np
import ml_dtypes

import concourse.bacc as bacc
import concourse.bass as bass
import concourse.mybir as mybir
import concourse.tile as tile
from concourse.bass_utils import run_bass_kernel_spmd
from concourse.library_config import mlp as mlp_lib

BF16 = ml_dtypes.bfloat16
AF = mybir.ActivationFunctionType
ALU = mybir.AluOpType

N_CORES = 8
N_NODES = 100000
NQ = 4               # table quarters (int16 index range)
NFEAT = 64
ROWPAD = 128         # table row width in bf16 elems (256 B)
NCLASS = 40
BN_EPS = 1e-5
SUPER = 512          # psum supertile width (nodes)
SHARD = N_NODES // N_CORES
HALF_A = 6656        # = 13 supertiles = 52 node-tiles, per core
HALF_B = SHARD - HALF_A
TAB_A = N_CORES * HALF_A    # 53248 rows
TAB_B = N_CORES * HALF_B    # 46752 rows
QSZ_A = TAB_A // 2          # 26624
QSZ_B = TAB_B // 2          # 23376
SUP_A = HALF_A // SUPER     # 13 supertiles in chunk A


def _bf16(x):
    return np.ascontiguousarray(np.asarray(x).astype(BF16))


def _f32(x):
    return np.ascontiguousarray(np.asarray(x, dtype=np.float32))


def _rowmap(src):
    """Global node id -> (quarter t, in-quarter row idx)."""
    src = np.asarray(src)
    c = src // SHARD
    i = src % SHARD
    in_a = i < HALF_A
    ra = c * HALF_A + i           # row within table A
    rb = c * HALF_B + (i - HALF_A)
    t = np.where(in_a, ra // QSZ_A, 2 + rb // QSZ_B)
    idx = np.where(in_a, ra % QSZ_A, rb % QSZ_B)
    return t.astype(np.int64), idx.astype(np.int64)


# ----------------------------------------------------------------------
# Host preprocessing
# ----------------------------------------------------------------------

class Structure:
    pass


def build_structure(edge_src, edge_dst, edge_w, n_nodes, n_cores):
    """Partition edges by dst shard and src table-quarter, dst-sort, build
    the shared column structure plus per-core gather-index / S tensors.

    Slot order: [supertile s][quarter t][column k][slot]. Every column is
    padded to 128 slots.
    """
    shard = n_nodes // n_cores
    st = Structure()
    st.shard = shard
    nsup = math.ceil(shard / SUPER)
    st.nsup = nsup
    st.sup_bounds = [(s * SUPER, min((s + 1) * SUPER, shard)) for s in range(nsup)]

    src_t, src_idx = _rowmap(edge_src)

    # per core, per quarter: dst-sorted edge lists + per-node degs
    deg = np.zeros((n_cores, NQ, shard), np.int64)
    edata = [[None] * NQ for _ in range(n_cores)]
    for c in range(n_cores):
        mc = (edge_dst // shard) == c
        rowc = src_idx[mc]
        tqc = src_t[mc]
        dstc = edge_dst[mc] - c * shard
        wc = edge_w[mc]
        for t in range(NQ):
            m = tqc == t
            row, dst, w = rowc[m], dstc[m], wc[m]
            o = np.argsort(dst, kind="stable")
            row, dst, w = row[o], dst[o], w[o]
            deg[c, t] = np.bincount(dst, minlength=shard)
            edata[c][t] = (row, dst, w)

    dcum = np.zeros((n_cores, NQ, shard + 1), np.int64)
    dcum[:, :, 1:] = np.cumsum(deg, axis=2)

    # Per-core greedy packing of nodes into 128-slot columns, then
    # union spans across cores (per-core placement is data; only span
    # geometry must be core-uniform). Empty trailing columns pad cores
    # that packed tighter.
    st.plan = {}      # (s,t) -> [(a, b)] union spans
    st.colnode = {}   # (s,t) -> per core: list of (a, b) actual node runs
    for s, (s0, s1) in enumerate(st.sup_bounds):
        for t in range(NQ):
            runs_pc = []
            for c in range(n_cores):
                runs = []
                a = s0
                while a < s1:
                    b = a + 1
                    while b < s1:
                        if dcum[c, t, b + 1] - dcum[c, t, a] > 128:
                            break
                        b += 1
                    runs.append((a, b))
                    a = b
                runs_pc.append(runs)
            ncolst = max(len(r) for r in runs_pc)
            for r in runs_pc:
                while len(r) < ncolst:
                    r.append((s1, s1))  # empty pad column
            spans = []
            for k in range(ncolst):
                a = min(r[k][0] for r in runs_pc)
                b = max(r[k][1] for r in runs_pc)
                spans.append((a, max(b, a + 1)))
            st.plan[(s, t)] = spans
            st.colnode[(s, t)] = runs_pc

    # layout: slots + S widths (S column widths = union spans, packed)
    st.order = [(s, t) for s in range(nsup) for t in range(NQ)]
    st.col_off = {}   # (s,t) -> column index offset
    st.s_off = {}     # (s,t) -> S free-dim offset
    st.sw = {}        # (s,t) -> total S width
    ncol = 0
    sumw = 0
    for (s, t) in st.order:
        st.col_off[(s, t)] = ncol
        st.s_off[(s, t)] = sumw
        w = sum(b - a for (a, b) in st.plan[(s, t)])
        st.sw[(s, t)] = w
        ncol += len(st.plan[(s, t)])
        sumw += w
    st.ncol = ncol
    st.sumw = sumw
    totslot = ncol * 128
    st.totslot = totslot

    idx_all, sv_all = [], []
    for c in range(n_cores):
        idxf = np.zeros(totslot, np.int16)
        S = np.zeros((128, sumw), np.float32)
        for (s, t) in st.order:
            row, dst, w = edata[c][t]
            cum = dcum[c, t]
            co = st.col_off[(s, t)]
            so = st.s_off[(s, t)]
            spans = st.plan[(s, t)]
            runs = st.colnode[(s, t)][c]
            woff = 0
            for k, (ua, ub) in enumerate(spans):
                a, b = runs[k]
                e0, e1 = int(cum[a]), int(cum[b])
                n = e1 - e0
                assert n <= 128
                base = (co + k) * 128
                idxf[base : base + n] = row[e0:e1].astype(np.int16)
                # S columns for this matmul = union span [ua, ub)
                S[np.arange(n), so + woff + (dst[e0:e1] - ua)] = w[e0:e1]
                woff += ub - ua
        # wrap: idx i -> [i%16 (replicated to 8 groups), i//16]
        iw = idxf.reshape(-1, 16).T  # [16, totslot/16]
        idx16 = np.tile(iw, (8, 1))  # [128, totslot/16]
        idx_all.append(np.ascontiguousarray(idx16))
        sv_all.append(_bf16(S))
    st.fill = float(edge_src.size / (n_cores * totslot))
    return st, idx_all, sv_all



def _dma_gather_raw(g, out_ap, in_ap, idxs_ap, num_idxs, elem_size, elem_step,
                    queue_num=0):
    """bass.dma_gather minus the elem_size%256 assert: 128-B payload descs
    from 256-B-stride table rows (stride_bytes_256 encodes the stride)."""
    stride_bytes = elem_step * mybir.dt.size(in_ap.dtype)
    _in_ap = g.lower_ap_dma(in_ap, for_custom_bir_dma=True)
    _idxs_ap = g.lower_ap(idxs_ap)
    _out_ap = g.lower_ap(out_ap)
    return g.add_instruction(
        mybir.InstDMAGatherAnt(
            name=g.bass.get_next_instruction_name(),
            ins=[*_in_ap, _idxs_ap, g.lower_val_access(g.to_reg(num_idxs))],
            outs=[_out_ap],
            transpose=False, num_idxs=num_idxs, elem_size=elem_size,
            stride_bytes_256=stride_bytes // 256,
            gen_mode=0, single_packet=False, queue_num=queue_num,
            sbuf_tokens_per_rank=0, sbuf_free_dim_per_rank=0,
            sbuf_free_dim_pad_per_rank=0, sbuf_byte_offset=0,
        ))


# ----------------------------------------------------------------------
# Bass program
# ----------------------------------------------------------------------

def build_program(st, n_nodes, n_cores, feats=(64, 64, 64, 128), profile=False):
    shard = st.shard
    nsup = st.nsup
    dt = mybir.dt

    nc = bacc.Bacc(
        "TRN2", target_bir_lowering=False, debug=False,
        num_devices=1 if profile else n_cores,
        dynamic_dma_scratch_size=65536,
        num_swdge_queues=4,
    )
    rg = [list(range(n_cores))]

    # ---------------- I/O ----------------
    idx_d = nc.dram_tensor(
        "idx", [128, st.totslot // 16], dt.int16, kind="ExternalInput"
    )
    sv_d = nc.dram_tensor("sv", [128, st.sumw], dt.bfloat16, kind="ExternalInput")
    wl_d = [
        nc.dram_tensor(f"wl{l}", [NFEAT, feats[l]], dt.bfloat16, kind="ExternalInput")
        for l in range(4)
    ]
    bl_d = [
        nc.dram_tensor(f"bl{l}", [feats[l], 1], dt.float32, kind="ExternalInput")
        for l in range(4)
    ]
    wcb_d = nc.dram_tensor("wcb", [128, 3 * 1024], dt.bfloat16, kind="ExternalInput")
    bcb_d = nc.dram_tensor("bcb", [128, 8], dt.float32, kind="ExternalInput")
    fw1_d = nc.dram_tensor("fw1", [128, 8 * 512], dt.bfloat16, kind="ExternalInput")
    fb1_d = nc.dram_tensor("fb1", [128, 4], dt.float32, kind="ExternalInput")
    fw2_d = nc.dram_tensor("fw2", [128, 4 * 256], dt.bfloat16, kind="ExternalInput")
    fb2_d = nc.dram_tensor("fb2", [128, 2], dt.float32, kind="ExternalInput")
    fw3_d = nc.dram_tensor("fw3", [128, 2 * NCLASS], dt.bfloat16, kind="ExternalInput")
    fb3_d = nc.dram_tensor("fb3", [NCLASS, 1], dt.float32, kind="ExternalInput")
    identp_d = nc.dram_tensor(
        "identp", [NFEAT, ROWPAD], dt.bfloat16, kind="ExternalInput"
    )
    out_d = nc.dram_tensor("out", [1, NCLASS], dt.float32, kind="ExternalOutput")

    # ---------------- tables / staging ----------------
    tabA_d = [None] * 4
    tabB_d = [None] * 4
    tabA_d[0] = nc.dram_tensor("table0a", [TAB_A, ROWPAD], dt.bfloat16,
                               kind="ExternalInput")
    tabB_d[0] = nc.dram_tensor("table0b", [TAB_B, ROWPAD], dt.bfloat16,
                               kind="ExternalInput")
    for l in range(1, 4):
        if profile:
            tabA_d[l] = nc.dram_tensor(f"table{l}a", [TAB_A, ROWPAD], dt.bfloat16,
                                       kind="ExternalInput")
            tabB_d[l] = nc.dram_tensor(f"table{l}b", [TAB_B, ROWPAD], dt.bfloat16,
                                       kind="ExternalInput")
        else:
            tabA_d[l] = nc.dram_tensor(f"table{l}a", [TAB_A, ROWPAD], dt.bfloat16,
                                       addr_space="Shared")
            tabB_d[l] = nc.dram_tensor(f"table{l}b", [TAB_B, ROWPAD], dt.bfloat16,
                                       addr_space="Shared")
    hA_d = [None] + [nc.dram_tensor(f"hA{l}", [HALF_A, ROWPAD], dt.bfloat16)
                     for l in range(1, 4)]
    hB_d = [None] + [nc.dram_tensor(f"hB{l}", [HALF_B, ROWPAD], dt.bfloat16)
                     for l in range(1, 4)]
    xiT_d = [
        nc.dram_tensor(f"xiT{l}", [feats[l], shard], dt.bfloat16) for l in range(3)
    ]
    cmax_d = nc.dram_tensor("cmax", [128, 8], dt.float32)
    gmax_d = nc.dram_tensor("gmax", [128, 8], dt.float32, addr_space="Shared")

    max_cols = max(len(st.plan[k]) for k in st.order)
    max_sw = max(st.sw[k] for k in st.order)

    def quarter_ap(l, t):
        if t < 2:
            return tabA_d[l][t * QSZ_A : (t + 1) * QSZ_A, 0:NFEAT]
        return tabB_d[l][(t - 2) * QSZ_B : (t - 1) * QSZ_B, 0:NFEAT]

    from contextlib import ExitStack

    with tile.TileContext(nc) as tc, ExitStack() as stk:
        cpool = stk.enter_context(tc.tile_pool(name="consts", bufs=1))
        xT_pool = stk.enter_context(tc.tile_pool(name="xT", bufs=1))
        small_pool = stk.enter_context(tc.tile_pool(name="small", bufs=1))
        conv_stk = ExitStack()
        msg_pool = conv_stk.enter_context(tc.tile_pool(name="msg", bufs=8))
        sv_pool = conv_stk.enter_context(tc.tile_pool(name="svp", bufs=6))
        ix_pool = conv_stk.enter_context(tc.tile_pool(name="ixp", bufs=6))
        agg_pool = conv_stk.enter_context(tc.tile_pool(name="aggsb", bufs=2))
        xnode_pool = conv_stk.enter_context(tc.tile_pool(name="xnode", bufs=3))
        comb_pool = conv_stk.enter_context(tc.tile_pool(name="comb", bufs=2))
        ps_agg = conv_stk.enter_context(
            tc.tile_pool(name="ps_agg", bufs=2, space="PSUM"))
        ps_x = conv_stk.enter_context(
            tc.tile_pool(name="ps_x", bufs=2, space="PSUM"))
        ps_t = conv_stk.enter_context(
            tc.tile_pool(name="ps_t", bufs=2, space="PSUM"))
        ps_c = conv_stk.enter_context(
            tc.tile_pool(name="ps_c", bufs=2, space="PSUM"))

        nc.gpsimd.load_library(mlp_lib)

        # ---- load constants ----
        wl_sb, bl_sb = [], []
        for l in range(4):
            w = cpool.tile([NFEAT, feats[l]], dt.bfloat16, tag=f"wl{l}")
            nc.sync.dma_start(out=w[:], in_=wl_d[l][:, :])
            b = cpool.tile([feats[l], 1], dt.float32, tag=f"bl{l}")
            nc.sync.dma_start(out=b[:], in_=bl_d[l][:, :])
            wl_sb.append(w)
            bl_sb.append(b)
        wcb_sb = cpool.tile([128, 3 * 1024], dt.bfloat16, tag="wcb")
        nc.sync.dma_start(out=wcb_sb[:], in_=wcb_d[:, :])
        bcb_sb = cpool.tile([128, 8], dt.float32, tag="bcb")
        nc.sync.dma_start(out=bcb_sb[:], in_=bcb_d[:, :])
        fw1_sb = cpool.tile([128, 8 * 512], dt.bfloat16, tag="fw1")
        nc.sync.dma_start(out=fw1_sb[:], in_=fw1_d[:, :])
        fb1_sb = cpool.tile([128, 4], dt.float32, tag="fb1")
        nc.sync.dma_start(out=fb1_sb[:], in_=fb1_d[:, :])
        fw2_sb = cpool.tile([128, 4 * 256], dt.bfloat16, tag="fw2")
        nc.sync.dma_start(out=fw2_sb[:], in_=fw2_d[:, :])
        fb2_sb = cpool.tile([128, 2], dt.float32, tag="fb2")
        nc.sync.dma_start(out=fb2_sb[:], in_=fb2_d[:, :])
        fw3_sb = cpool.tile([128, 2 * NCLASS], dt.bfloat16, tag="fw3")
        nc.sync.dma_start(out=fw3_sb[:], in_=fw3_d[:, :])
        fb3_sb = cpool.tile([NCLASS, 1], dt.float32, tag="fb3")
        nc.sync.dma_start(out=fb3_sb[:], in_=fb3_d[:, :])
        identp_sb = cpool.tile([NFEAT, ROWPAD], dt.bfloat16, tag="identp")
        nc.sync.dma_start(out=identp_sb[:], in_=identp_d[:, :])
        zero64_sb = cpool.tile([128, NFEAT], dt.bfloat16, tag="zero64")
        nc.gpsimd.memset(zero64_sb[:], 0.0)
        cmax_sb = small_pool.tile([128, 8], dt.float32, tag="cmax")

        ntile_tr = math.ceil(shard / 128)

        for l in range(4):
            nf = feats[l]
            xT_full = xT_pool.tile([128, shard], dt.bfloat16, tag="xT")
            xT_sb = xT_full[:nf, :]
            for s in range(nsup):
                s0, s1 = st.sup_bounds[s]
                sw = s1 - s0
                pagg = ps_agg.tile([64, SUPER], dt.float32, tag="pagg")
                first_mm = True
                for t in range(NQ):
                    colspans = st.plan[(s, t)]
                    ncols = len(colspans)
                    nidx = ncols * 128
                    co = st.col_off[(s, t)]
                    so = st.s_off[(s, t)]
                    # stream idx + S for this (s, t)
                    ix = ix_pool.tile([128, max_cols * 8], dt.int16, tag="ix")
                    nc.sync.dma_start(
                        out=ix[:, : nidx // 16],
                        in_=idx_d[:, co * 8 : co * 8 + nidx // 16],
                    )
                    swst = st.sw[(s, t)]
                    sv = sv_pool.tile([128, max_sw], dt.bfloat16, tag="sv")
                    nc.sync.dma_start(
                        out=sv[:, :swst], in_=sv_d[:, so : so + swst]
                    )
                    msg = msg_pool.tile(
                        [128, max_cols * NFEAT], dt.bfloat16, tag="msg"
                    )
                    _dma_gather_raw(
                        nc.gpsimd,
                        msg[:, : ncols * NFEAT].rearrange(
                            "p (c e) -> p c e", e=NFEAT
                        ),
                        quarter_ap(l, t),
                        ix[:, : nidx // 16],
                        nidx,
                        NFEAT,
                        ROWPAD,
                        queue_num=(s + t) % 4,
                    )
                    if first_mm:
                        # full-width zeroing matmul: start=True clears the
                        # bank and gives every data matmul a WAW dep on it
                        # (data matmuls then accumulate in any order).
                        nc.tensor.matmul(
                            out=pagg[:, :sw],
                            lhsT=zero64_sb[:, :],
                            rhs=sv[:, :sw],
                            start=True, stop=False,
                            skip_group_check=True,
                        )
                        first_mm = False
                    woff = 0
                    for k, (a, b) in enumerate(colspans):
                        nc.tensor.matmul(
                            out=pagg[:, a - s0 : b - s0],
                            lhsT=msg[:, k * NFEAT : (k + 1) * NFEAT],
                            rhs=sv[:, woff : woff + (b - a)],
                            start=False,
                            stop=(t == NQ - 1 and k == len(colspans) - 1),
                            skip_group_check=True,
                        )
                        woff += b - a
                agg_sb = agg_pool.tile([64, SUPER], dt.bfloat16, tag="aggsb")
                nc.vector.tensor_copy(out=agg_sb[:, :sw], in_=pagg[:, :sw])
                pxt = ps_x.tile([128, SUPER], dt.float32, tag="px")
                px = pxt[:nf, :]
                nc.tensor.matmul(
                    out=px[:, :sw],
                    lhsT=wl_sb[l][:, :],
                    rhs=agg_sb[:, :sw],
                    start=True,
                    stop=True,
                )
                nc.scalar.activation(
                    out=xT_sb[:, s0:s1],
                    in_=px[:, :sw],
                    func=AF.Relu,
                    bias=bl_sb[l][:, :],
                    scale=1.0,
                )
                if l < 3:
                    # node-major (row-padded) transpose of this supertile
                    # via TensorE, staged to hA/hB for the chunked AG
                    t0i = (s0 // 128)
                    t1i = math.ceil(s1 / 128)
                    nt_s = t1i - t0i
                    xnode = xnode_pool.tile(
                        [128, 4 * ROWPAD], dt.bfloat16, tag="xnode"
                    )
                    for j in range(nt_s):
                        n0 = (t0i + j) * 128
                        n1 = min(n0 + 128, shard)
                        pt = ps_t.tile([128, ROWPAD], dt.float32, tag="pt")
                        nc.tensor.matmul(
                            out=pt[: n1 - n0, :],
                            lhsT=xT_sb[:, n0:n1],
                            rhs=identp_sb[:, :],
                            start=True,
                            stop=True,
                        )
                        nc.vector.tensor_copy(
                            out=xnode[: n1 - n0, j * ROWPAD : (j + 1) * ROWPAD],
                            in_=pt[: n1 - n0, :],
                        )
                    rows = s1 - s0
                    nfull = rows // 128
                    rem = rows % 128
                    if s < SUP_A:
                        dst, off = hA_d[l + 1], s0
                    else:
                        dst, off = hB_d[l + 1], s0 - HALF_A
                    if nfull:
                        nc.sync.dma_start(
                            out=dst[off : off + nfull * 128, :].rearrange(
                                "(t p) f -> p t f", p=128
                            ),
                            in_=xnode[:, : nfull * ROWPAD].rearrange(
                                "p (t f) -> p t f", f=ROWPAD
                            ),
                        )
                    if rem:
                        nc.sync.dma_start(
                            out=dst[off + nfull * 128 : off + rows, :],
                            in_=xnode[:rem, nfull * ROWPAD : (nfull + 1) * ROWPAD],
                        )
                if l < 3 and s == SUP_A + 1 and not profile:
                    # chunk-A AllGather: overlaps remaining supertiles
                    nc.gpsimd.collective_compute(
                        "AllGather", ALU.bypass, replica_groups=rg,
                        ins=[hA_d[l + 1][:, :]], outs=[tabA_d[l + 1][:, :]],
                    )
                if l == 3:
                    # fused 320->1024 combination + max-pool for this chunk
                    q0 = comb_pool.tile([128, SUPER], dt.bfloat16, tag="q0")
                    nc.sync.dma_start(out=q0[0:64, :sw], in_=xiT_d[0][:, s0:s1])
                    nc.sync.dma_start(out=q0[64:128, :sw], in_=xiT_d[1][:, s0:s1])
                    q1 = comb_pool.tile([64, SUPER], dt.bfloat16, tag="q1")
                    nc.sync.dma_start(out=q1[:, :sw], in_=xiT_d[2][:, s0:s1])
                    for m in range(8):
                        pc = ps_c.tile([128, SUPER], dt.float32, tag="pc")
                        nc.tensor.matmul(
                            out=pc[:, :sw],
                            lhsT=wcb_sb[0:128, m * 128 : (m + 1) * 128],
                            rhs=q0[:, :sw],
                            start=True, stop=False,
                        )
                        nc.tensor.matmul(
                            out=pc[:, :sw],
                            lhsT=wcb_sb[0:64, 1024 + m * 128 : 1024 + (m + 1) * 128],
                            rhs=q1[:, :sw],
                            start=False, stop=False,
                        )
                        nc.tensor.matmul(
                            out=pc[:, :sw],
                            lhsT=wcb_sb[0:128, 2048 + m * 128 : 2048 + (m + 1) * 128],
                            rhs=xT_full[:, s0:s1],
                            start=False, stop=True,
                        )
                        red = small_pool.tile([128, 1], dt.float32, tag="red")
                        nc.vector.tensor_reduce(
                            out=red[:, :],
                            in_=pc[:, :sw],
                            axis=mybir.AxisListType.X,
                            op=ALU.max,
                        )
                        if s == 0:
                            nc.vector.tensor_copy(
                                out=cmax_sb[:, m : m + 1], in_=red[:, :]
                            )
                        else:
                            nc.vector.tensor_tensor(
                                out=cmax_sb[:, m : m + 1],
                                in0=cmax_sb[:, m : m + 1],
                                in1=red[:, :],
                                op=ALU.max,
                            )
            if l < 3:
                if not profile:
                    nc.gpsimd.collective_compute(
                        "AllGather", ALU.bypass, replica_groups=rg,
                        ins=[hB_d[l + 1][:, :]], outs=[tabB_d[l + 1][:, :]],
                    )
                # spill xT for the combination stage
                nc.sync.dma_start(out=xiT_d[l][:, :], in_=xT_sb[:, :])

        # ---------------- all-reduce max + head ----------------
        conv_stk.close()
        ps_h = stk.enter_context(tc.tile_pool(name="ps_h", bufs=2, space="PSUM"))
        if profile:
            gmax_sb = cmax_sb
        else:
            nc.sync.dma_start(out=cmax_d[:, :], in_=cmax_sb[:, :])
            nc.gpsimd.collective_compute(
                "AllReduce", ALU.max, replica_groups=rg,
                ins=[cmax_d[:, :]], outs=[gmax_d[:, :]],
            )
            gmax_sb = small_pool.tile([128, 8], dt.float32, tag="gmax")
            nc.sync.dma_start(out=gmax_sb[:, :], in_=gmax_d[:, :])
        h0 = small_pool.tile([128, 8], dt.bfloat16, tag="h0")
        hpre = small_pool.tile([128, 8], dt.float32, tag="hpre")
        nc.vector.tensor_tensor(
            out=hpre[:, :], in0=gmax_sb[:, :], in1=bcb_sb[:, :], op=ALU.add
        )
        nc.vector.tensor_relu(out=h0[:, :], in_=hpre[:, :])

        h1 = small_pool.tile([128, 4], dt.bfloat16, tag="h1")
        for o in range(4):
            ph = ps_h.tile([128, 1], dt.float32, tag="ph")
            for q in range(8):
                nc.tensor.matmul(
                    out=ph[:, :],
                    lhsT=fw1_sb[:, q * 512 + o * 128 : q * 512 + (o + 1) * 128],
                    rhs=h0[:, q : q + 1],
                    start=(q == 0), stop=(q == 7),
                )
            nc.scalar.activation(
                out=h1[:, o : o + 1], in_=ph[:, :], func=AF.Relu,
                bias=fb1_sb[:, o : o + 1], scale=1.0,
            )
        h2 = small_pool.tile([128, 2], dt.bfloat16, tag="h2")
        for o in range(2):
            ph = ps_h.tile([128, 1], dt.float32, tag="ph")
            for q in range(4):
                nc.tensor.matmul(
                    out=ph[:, :],
                    lhsT=fw2_sb[:, q * 256 + o * 128 : q * 256 + (o + 1) * 128],
                    rhs=h1[:, q : q + 1],
                    start=(q == 0), stop=(q == 3),
                )
            nc.scalar.activation(
                out=h2[:, o : o + 1], in_=ph[:, :], func=AF.Relu,
                bias=fb2_sb[:, o : o + 1], scale=1.0,
            )
        po = ps_h.tile([NCLASS, 1], dt.float32, tag="po")
        for q in range(2):
            nc.tensor.matmul(
                out=po[:, :],
                lhsT=fw3_sb[:, q * NCLASS : (q + 1) * NCLASS],
                rhs=h2[:, q : q + 1],
                start=(q == 0), stop=(q == 1),
            )
        out_sb = small_pool.tile([NCLASS, 1], dt.float32, tag="outsb")
        nc.vector.tensor_tensor(
            out=out_sb[:, :], in0=po[:, :], in1=fb3_sb[:, :], op=ALU.add
        )
        nc.sync.dma_start(
            out=out_d.ap().rearrange("a b -> b a"), in_=out_sb[:, :]
        )

    nc.compile()
    return nc


# ----------------------------------------------------------------------
# Host wrapper
# ----------------------------------------------------------------------

def make_inputs(inputs, st, idx_all, sv_all, n_nodes, n_cores):
    shard = n_nodes // n_cores
    x = _f32(inputs["x"])
    s_bn = lambda g: _f32(g) / np.sqrt(np.float32(1.0) + np.float32(BN_EPS))

    def pack_lhsT(w, kdim, mdim):  # w: [out, in] -> [128, (kdim/128)*mdim]
        nq = kdim // 128
        arr = np.zeros((128, nq * mdim), np.float32)
        for q in range(nq):
            arr[:, q * mdim : (q + 1) * mdim] = w[:, q * 128 : (q + 1) * 128].T
        return _bf16(arr)

    s1, s2 = s_bn(inputs["g1"]), s_bn(inputs["g2"])
    w1 = _f32(inputs["fc1_w"]) * s1[:, None]
    b1 = _f32(inputs["fc1_b"]) * s1 + _f32(inputs["be1"])
    w2 = _f32(inputs["fc2_w"]) * s2[:, None]
    b2 = _f32(inputs["fc2_b"]) * s2 + _f32(inputs["be2"])
    w3 = _f32(inputs["fc3_w"])
    b3 = _f32(inputs["fc3_b"])

    fw1 = pack_lhsT(w1, 1024, 512)
    fb1 = np.zeros((128, 4), np.float32)
    for o in range(4):
        fb1[:, o] = b1[o * 128 : (o + 1) * 128]
    fw2 = pack_lhsT(w2, 512, 256)
    fb2 = np.zeros((128, 2), np.float32)
    for o in range(2):
        fb2[:, o] = b2[o * 128 : (o + 1) * 128]
    fw3 = pack_lhsT(w3, 256, NCLASS)
    fb3 = b3[:, None]

    wcb = _f32(inputs["Wcb"])  # [320, 1024]
    wcb_p = np.zeros((128, 3 * 1024), np.float32)
    wcb_p[0:128, 0:1024] = wcb[0:128]       # vs q0 = [x1; x2]
    wcb_p[0:64, 1024:2048] = wcb[128:192]   # vs q1 = [x3]
    wcb_p[0:128, 2048:3072] = wcb[192:320]  # vs x4 (straight from SBUF)
    bcb = _f32(inputs["bcb"])
    bcb_p = np.zeros((128, 8), np.float32)
    for m in range(8):
        bcb_p[:, m] = bcb[m * 128 : (m + 1) * 128]

    identp = np.zeros((NFEAT, ROWPAD), np.float32)
    identp[:, :NFEAT] = np.eye(NFEAT)

    # full permuted layer-0 table, replicated on every core
    nodes = np.arange(n_nodes)
    tq, tidx = _rowmap(nodes)
    rowsA = np.where(tq < 2, tq * QSZ_A + tidx, 0)
    rowsB = np.where(tq >= 2, (tq - 2) * QSZ_B + tidx, 0)
    xa = np.zeros((TAB_A, ROWPAD), np.float32)
    xb = np.zeros((TAB_B, ROWPAD), np.float32)
    ma = tq < 2
    xa[rowsA[ma], :NFEAT] = x[nodes[ma]]
    xb[rowsB[~ma], :NFEAT] = x[nodes[~ma]]

    common = dict(
        wcb=_bf16(wcb_p),
        bcb=bcb_p,
        fw1=fw1, fb1=fb1, fw2=fw2, fb2=fb2, fw3=fw3, fb3=fb3,
        identp=_bf16(identp),
        table0a=_bf16(xa), table0b=_bf16(xb),
    )
    for l in range(4):
        common[f"wl{l}"] = _bf16(inputs[f"W{l + 1}"])
        common[f"bl{l}"] = _f32(inputs[f"b{l + 1}"])[:, None]

    in_maps = []
    for c in range(n_cores):
        m = dict(common)
        m["sv"] = sv_all[c]
        m["idx"] = idx_all[c]
        in_maps.append(m)
    return in_maps


LAST_EXEC_NS = None


def _install_ntff_hook():
    """Best-effort shim for antenv.axon_hooks (absent on some images) so
    run_bass_kernel_spmd(trace=True) can capture NTFF profiles."""
    import sys
    import types
    try:
        from antenv.axon_hooks import get_axon_ntff_profile_hook  # noqa: F401
        return
    except ImportError:
        pass
    try:
        import antenv
        from trn_agent_boot.trn_boot import _ntff_profile_via_ctypes
        mod = types.ModuleType("antenv.axon_hooks")
        _hook = [None]
        mod.set_axon_ntff_profile_hook = lambda h: _hook.__setitem__(0, h)
        mod.get_axon_ntff_profile_hook = lambda: _hook[0]
        antenv.axon_hooks = mod
        sys.modules["antenv.axon_hooks"] = mod
        mod.set_axon_ntff_profile_hook(
            _ntff_profile_via_ctypes("/opt/axon/libaxon_pjrt.so")
        )
    except Exception:
        pass


def kernel(**inputs):
    global LAST_EXEC_NS
    import os
    import time

    edge_src = np.asarray(inputs["edge_src"])
    edge_dst = np.asarray(inputs["edge_dst"])
    edge_w = _f32(inputs["edge_w"])

    t0 = time.time()
    st, idx_all, sv_all = build_structure(
        edge_src, edge_dst, edge_w, N_NODES, N_CORES
    )
    t1 = time.time()
    nc = build_program(st, N_NODES, N_CORES)
    t2 = time.time()
    in_maps = make_inputs(inputs, st, idx_all, sv_all, N_NODES, N_CORES)
    trace = bool(int(os.environ.get("KERNEL_TRACE", "0")))
    if trace:
        _install_ntff_hook()
    res = run_bass_kernel_spmd(
        nc, in_maps, core_ids=list(range(N_CORES)), trace=trace
    )
    t3 = time.time()
    LAST_EXEC_NS = res.exec_time_ns
    print(
        f"[kernel] fill={st.fill:.3f} ncol={st.ncol} "
        f"prep={t1 - t0:.1f}s build+compile={t2 - t1:.1f}s "
        f"run={t3 - t2:.1f}s exec_ns={res.exec_time_ns}"
    )
    return np.asarray(res.results[0]["out"], dtype=np.float32)


if __name__ == "__main__":
    data = dict(np.load("/root/problem/inputs.npz"))
    out = kernel(**data)
    print(out)


# revision 17
# speedup vs baseline: 1.1058x; 1.1058x over previous
"""Trainium2 Bass kernel for nn_MeshNetV0 (GNN message passing), 8 cores.

Strategy (pull-mode sharding):
  - Core c owns dst nodes [c*SHARD, (c+1)*SHARD) and all their in-edges.
  - Layer-0 node features are fed pre-replicated as full-table inputs
    (tableA/B), so no layer-0 AllGather is needed.
  - Per conv layer: gpsimd dma_gather pulls per-edge source rows (bf16,
    256 B stride) into edge-major SBUF tiles [128 slots, cols, 64]; per
    128-slot column a TensorE matmul  msg[:, :64].T @ S  (S = host-built
    weighted one-hot [128, w], bf16) computes the weighted segment-sum
    into feature-major PSUM [64, node-window]; the @W runs
    post-aggregation ((A h) W) and ACT applies bias+ReLU on the drain.
  - Gathers are spread over all 4 SWDGE queues (rotating with s so the
    unequal quarter sizes average out) — desc generation runs on 4 Q7
    core pairs concurrently.
  - Node features for the next layer are re-transposed per supertile and
    staged to DRAM in two chunks (A = supertiles 0-12, B = 13-24); each
    chunk AllGathers separately so chunk A's collective overlaps the
    tail of the current layer's gathers.
  - The int16 gather indices address four table quarters: A is split in
    2x 26624 rows, B in 2x 23376 rows.
  - The 320->1024 combination + max-pool run fused into layer 3's
    supertile loop (x4 read straight from SBUF); AllReduce-max merges;
    the BN-folded FC head is replicated on every core.

The column structure (node ranges per 128-slot column) is built from
max-over-cores degree sums so the instruction stream is identical on all
8 cores; only tensor contents (gather indices, S values) differ.
"""

import math
import numpy as np
import ml_dtypes

import concourse.bacc as bacc
import concourse.bass as bass
import concourse.mybir as mybir
import concourse.tile as tile
from concourse.bass_utils import run_bass_kernel_spmd
from concourse.library_config import mlp as mlp_lib

BF16 = ml_dtypes.bfloat16
AF = mybir.ActivationFunctionType
ALU = mybir.AluOpType

N_CORES = 8
N_NODES = 100000
NQ = 4               # table quarters (int16 index range)
NFEAT = 64
ROWPAD = 128         # table row width in bf16 elems (256 B)
NCLASS = 40
BN_EPS = 1e-5
SUPER = 512          # psum supertile width (nodes)
SHARD = N_NODES // N_CORES
HALF_A = 6656        # = 13 supertiles (s 0-12), per core
HALF_B = 3584        # supertiles 13-19
HALF_C = SHARD - HALF_A - HALF_B   # 2260, supertiles 20-24
TAB_A = N_CORES * HALF_A    # 53248 rows (2 int16 quarters)
TAB_B = N_CORES * HALF_B    # 28672 rows (quarter 2)
TAB_C = N_CORES * HALF_C    # 18080 rows (quarter 3)
QSZ_A = TAB_A // 2          # 26624
SUP_A = HALF_A // SUPER     # 13 supertiles in chunk A
SUP_B = (HALF_A + HALF_B) // SUPER  # 20: first supertile of chunk C


def _bf16(x):
    return np.ascontiguousarray(np.asarray(x).astype(BF16))


def _f32(x):
    return np.ascontiguousarray(np.asarray(x, dtype=np.float32))


def _rowmap(src):
    """Global node id -> (quarter t, in-quarter row idx)."""
    src = np.asarray(src)
    c = src // SHARD
    i = src % SHARD
    ra = c * HALF_A + i                       # row within table A
    rb = c * HALF_B + (i - HALF_A)            # row within table B
    rc = c * HALF_C + (i - HALF_A - HALF_B)   # row within table C
    in_a = i < HALF_A
    in_b = (~in_a) & (i < HALF_A + HALF_B)
    t = np.where(in_a, ra // QSZ_A, np.where(in_b, 2, 3))
    idx = np.where(in_a, ra % QSZ_A, np.where(in_b, rb, rc))
    return t.astype(np.int64), idx.astype(np.int64)


# ----------------------------------------------------------------------
# Host preprocessing
# ----------------------------------------------------------------------

class Structure:
    pass


def build_structure(edge_src, edge_dst, edge_w, n_nodes, n_cores):
    """Partition edges by dst shard and src table-quarter, dst-sort, build
    the shared column structure plus per-core gather-index / S tensors.

    Slot order: [supertile s][quarter t][column k][slot]. Every column is
    padded to 128 slots.
    """
    shard = n_nodes // n_cores
    st = Structure()
    st.shard = shard
    nsup = math.ceil(shard / SUPER)
    st.nsup = nsup
    st.sup_bounds = [(s * SUPER, min((s + 1) * SUPER, shard)) for s in range(nsup)]

    src_t, src_idx = _rowmap(edge_src)

    # per core, per quarter: dst-sorted edge lists + per-node degs
    deg = np.zeros((n_cores, NQ, shard), np.int64)
    edata = [[None] * NQ for _ in range(n_cores)]
    for c in range(n_cores):
        mc = (edge_dst // shard) == c
        rowc = src_idx[mc]
        tqc = src_t[mc]
        dstc = edge_dst[mc] - c * shard
        wc = edge_w[mc]
        for t in range(NQ):
            m = tqc == t
            row, dst, w = rowc[m], dstc[m], wc[m]
            o = np.argsort(dst, kind="stable")
            row, dst, w = row[o], dst[o], w[o]
            deg[c, t] = np.bincount(dst, minlength=shard)
            edata[c][t] = (row, dst, w)

    dcum = np.zeros((n_cores, NQ, shard + 1), np.int64)
    dcum[:, :, 1:] = np.cumsum(deg, axis=2)

    # Per-core greedy packing of nodes into 128-slot columns, then
    # union spans across cores (per-core placement is data; only span
    # geometry must be core-uniform). Empty trailing columns pad cores
    # that packed tighter.
    st.plan = {}      # (s,t) -> [(a, b)] union spans
    st.colnode = {}   # (s,t) -> per core: list of (a, b) actual node runs
    for s, (s0, s1) in enumerate(st.sup_bounds):
        for t in range(NQ):
            runs_pc = []
            for c in range(n_cores):
                runs = []
                a = s0
                while a < s1:
                    b = a + 1
                    while b < s1:
                        if dcum[c, t, b + 1] - dcum[c, t, a] > 128:
                            break
                        b += 1
                    runs.append((a, b))
                    a = b
                runs_pc.append(runs)
            ncolst = max(len(r) for r in runs_pc)
            for r in runs_pc:
                while len(r) < ncolst:
                    r.append((s1, s1))  # empty pad column
            spans = []
            for k in range(ncolst):
                a = min(r[k][0] for r in runs_pc)
                b = max(r[k][1] for r in runs_pc)
                spans.append((a, max(b, a + 1)))
            st.plan[(s, t)] = spans
            st.colnode[(s, t)] = runs_pc

    # layout: slots + S widths (S column widths = union spans, packed)
    st.order = [(s, t) for s in range(nsup) for t in range(NQ)]
    st.col_off = {}   # (s,t) -> column index offset
    st.s_off = {}     # (s,t) -> S free-dim offset
    st.sw = {}        # (s,t) -> total S width
    ncol = 0
    sumw = 0
    for (s, t) in st.order:
        st.col_off[(s, t)] = ncol
        st.s_off[(s, t)] = sumw
        w = sum(b - a for (a, b) in st.plan[(s, t)])
        st.sw[(s, t)] = w
        ncol += len(st.plan[(s, t)])
        sumw += w
    st.ncol = ncol
    st.sumw = sumw
    totslot = ncol * 128
    st.totslot = totslot

    idx_all, sv_all = [], []
    for c in range(n_cores):
        idxf = np.zeros(totslot, np.int16)
        S = np.zeros((128, sumw), np.float32)
        for (s, t) in st.order:
            row, dst, w = edata[c][t]
            cum = dcum[c, t]
            co = st.col_off[(s, t)]
            so = st.s_off[(s, t)]
            spans = st.plan[(s, t)]
            runs = st.colnode[(s, t)][c]
            woff = 0
            for k, (ua, ub) in enumerate(spans):
                a, b = runs[k]
                e0, e1 = int(cum[a]), int(cum[b])
                n = e1 - e0
                assert n <= 128
                base = (co + k) * 128
                idxf[base : base + n] = row[e0:e1].astype(np.int16)
                # S columns for this matmul = union span [ua, ub)
                S[np.arange(n), so + woff + (dst[e0:e1] - ua)] = w[e0:e1]
                woff += ub - ua
        # wrap: idx i -> [i%16 (replicated to 8 groups), i//16]
        iw = idxf.reshape(-1, 16).T  # [16, totslot/16]
        idx16 = np.tile(iw, (8, 1))  # [128, totslot/16]
        idx_all.append(np.ascontiguousarray(idx16))
        sv_all.append(_bf16(S))
    st.fill = float(edge_src.size / (n_cores * totslot))
    return st, idx_all, sv_all



def _dma_gather_raw(g, out_ap, in_ap, idxs_ap, num_idxs, elem_size, elem_step,
                    queue_num=0):
    """bass.dma_gather minus the elem_size%256 assert: 128-B payload descs
    from 256-B-stride table rows (stride_bytes_256 encodes the stride)."""
    stride_bytes = elem_step * mybir.dt.size(in_ap.dtype)
    _in_ap = g.lower_ap_dma(in_ap, for_custom_bir_dma=True)
    _idxs_ap = g.lower_ap(idxs_ap)
    _out_ap = g.lower_ap(out_ap)
    return g.add_instruction(
        mybir.InstDMAGatherAnt(
            name=g.bass.get_next_instruction_name(),
            ins=[*_in_ap, _idxs_ap, g.lower_val_access(g.to_reg(num_idxs))],
            outs=[_out_ap],
            transpose=False, num_idxs=num_idxs, elem_size=elem_size,
            stride_bytes_256=stride_bytes // 256,
            gen_mode=0, single_packet=False, queue_num=queue_num,
            sbuf_tokens_per_rank=0, sbuf_free_dim_per_rank=0,
            sbuf_free_dim_pad_per_rank=0, sbuf_byte_offset=0,
        ))


# ----------------------------------------------------------------------
# Bass program
# ----------------------------------------------------------------------

def build_program(st, n_nodes, n_cores, feats=(64, 64, 64, 128), profile=False):
    shard = st.shard
    nsup = st.nsup
    dt = mybir.dt

    nc = bacc.Bacc(
        "TRN2", target_bir_lowering=False, debug=False,
        num_devices=1 if profile else n_cores,
        dynamic_dma_scratch_size=65536,
        num_swdge_queues=4,
    )
    rg = [list(range(n_cores))]

    # ---------------- I/O ----------------
    idx_d = nc.dram_tensor(
        "idx", [128, st.totslot // 16], dt.int16, kind="ExternalInput"
    )
    sv_d = nc.dram_tensor("sv", [128, st.sumw], dt.bfloat16, kind="ExternalInput")
    wl_d = [
        nc.dram_tensor(f"wl{l}", [NFEAT, feats[l]], dt.bfloat16, kind="ExternalInput")
        for l in range(4)
    ]
    bl_d = [
        nc.dram_tensor(f"bl{l}", [feats[l], 1], dt.float32, kind="ExternalInput")
        for l in range(4)
    ]
    wcb_d = nc.dram_tensor("wcb", [128, 3 * 1024], dt.bfloat16, kind="ExternalInput")
    bcb_d = nc.dram_tensor("bcb", [128, 8], dt.float32, kind="ExternalInput")
    fw1_d = nc.dram_tensor("fw1", [128, 8 * 512], dt.bfloat16, kind="ExternalInput")
    fb1_d = nc.dram_tensor("fb1", [128, 4], dt.float32, kind="ExternalInput")
    fw2_d = nc.dram_tensor("fw2", [128, 4 * 256], dt.bfloat16, kind="ExternalInput")
    fb2_d = nc.dram_tensor("fb2", [128, 2], dt.float32, kind="ExternalInput")
    fw3_d = nc.dram_tensor("fw3", [128, 2 * NCLASS], dt.bfloat16, kind="ExternalInput")
    fb3_d = nc.dram_tensor("fb3", [NCLASS, 1], dt.float32, kind="ExternalInput")
    identp_d = nc.dram_tensor(
        "identp", [NFEAT, ROWPAD], dt.bfloat16, kind="ExternalInput"
    )
    out_d = nc.dram_tensor("out", [1, NCLASS], dt.float32, kind="ExternalOutput")

    # ---------------- tables / staging ----------------
    tabA_d = [None] * 4
    tabB_d = [None] * 4
    tabC_d = [None] * 4
    kinds = dict(kind="ExternalInput")
    shared = dict(kind="ExternalInput") if profile else dict(addr_space="Shared")
    tabA_d[0] = nc.dram_tensor("table0a", [TAB_A, ROWPAD], dt.bfloat16, **kinds)
    tabB_d[0] = nc.dram_tensor("table0b", [TAB_B, ROWPAD], dt.bfloat16, **kinds)
    tabC_d[0] = nc.dram_tensor("table0c", [TAB_C, ROWPAD], dt.bfloat16, **kinds)
    for l in range(1, 4):
        tabA_d[l] = nc.dram_tensor(f"table{l}a", [TAB_A, ROWPAD], dt.bfloat16,
                                   **shared)
        tabB_d[l] = nc.dram_tensor(f"table{l}b", [TAB_B, ROWPAD], dt.bfloat16,
                                   **shared)
        tabC_d[l] = nc.dram_tensor(f"table{l}c", [TAB_C, ROWPAD], dt.bfloat16,
                                   **shared)
    hA_d = [None] + [nc.dram_tensor(f"hA{l}", [HALF_A, ROWPAD], dt.bfloat16)
                     for l in range(1, 4)]
    hB_d = [None] + [nc.dram_tensor(f"hB{l}", [HALF_B, ROWPAD], dt.bfloat16)
                     for l in range(1, 4)]
    hC_d = [None] + [nc.dram_tensor(f"hC{l}", [HALF_C, ROWPAD], dt.bfloat16)
                     for l in range(1, 4)]
    xiT_d = [
        nc.dram_tensor(f"xiT{l}", [feats[l], shard], dt.bfloat16) for l in range(3)
    ]
    cmax_d = nc.dram_tensor("cmax", [128, 8], dt.float32)
    gmax_d = nc.dram_tensor("gmax", [128, 8], dt.float32, addr_space="Shared")

    max_cols = max(len(st.plan[k]) for k in st.order)
    max_sw = max(st.sw[k] for k in st.order)

    def quarter_ap(l, t):
        if t < 2:
            return tabA_d[l][t * QSZ_A : (t + 1) * QSZ_A, 0:NFEAT]
        if t == 2:
            return tabB_d[l][:, 0:NFEAT]
        return tabC_d[l][:, 0:NFEAT]

    from contextlib import ExitStack

    with tile.TileContext(nc) as tc, ExitStack() as stk:
        cpool = stk.enter_context(tc.tile_pool(name="consts", bufs=1))
        xT_pool = stk.enter_context(tc.tile_pool(name="xT", bufs=1))
        small_pool = stk.enter_context(tc.tile_pool(name="small", bufs=1))
        conv_stk = ExitStack()
        msg_pool = conv_stk.enter_context(tc.tile_pool(name="msg", bufs=10))
        sv_pool = conv_stk.enter_context(tc.tile_pool(name="svp", bufs=8))
        ix_pool = conv_stk.enter_context(tc.tile_pool(name="ixp", bufs=8))
        agg_pool = conv_stk.enter_context(tc.tile_pool(name="aggsb", bufs=2))
        xnode_pool = conv_stk.enter_context(tc.tile_pool(name="xnode", bufs=3))
        comb_pool = conv_stk.enter_context(tc.tile_pool(name="comb", bufs=2))
        ps_agg = conv_stk.enter_context(
            tc.tile_pool(name="ps_agg", bufs=2, space="PSUM"))
        ps_x = conv_stk.enter_context(
            tc.tile_pool(name="ps_x", bufs=2, space="PSUM"))
        ps_t = conv_stk.enter_context(
            tc.tile_pool(name="ps_t", bufs=2, space="PSUM"))
        ps_c = conv_stk.enter_context(
            tc.tile_pool(name="ps_c", bufs=2, space="PSUM"))

        nc.gpsimd.load_library(mlp_lib)

        # ---- load constants ----
        wl_sb, bl_sb = [], []
        for l in range(4):
            w = cpool.tile([NFEAT, feats[l]], dt.bfloat16, tag=f"wl{l}")
            nc.sync.dma_start(out=w[:], in_=wl_d[l][:, :])
            b = cpool.tile([feats[l], 1], dt.float32, tag=f"bl{l}")
            nc.sync.dma_start(out=b[:], in_=bl_d[l][:, :])
            wl_sb.append(w)
            bl_sb.append(b)
        wcb_sb = cpool.tile([128, 3 * 1024], dt.bfloat16, tag="wcb")
        nc.sync.dma_start(out=wcb_sb[:], in_=wcb_d[:, :])
        bcb_sb = cpool.tile([128, 8], dt.float32, tag="bcb")
        nc.sync.dma_start(out=bcb_sb[:], in_=bcb_d[:, :])
        fw1_sb = cpool.tile([128, 8 * 512], dt.bfloat16, tag="fw1")
        nc.sync.dma_start(out=fw1_sb[:], in_=fw1_d[:, :])
        fb1_sb = cpool.tile([128, 4], dt.float32, tag="fb1")
        nc.sync.dma_start(out=fb1_sb[:], in_=fb1_d[:, :])
        fw2_sb = cpool.tile([128, 4 * 256], dt.bfloat16, tag="fw2")
        nc.sync.dma_start(out=fw2_sb[:], in_=fw2_d[:, :])
        fb2_sb = cpool.tile([128, 2], dt.float32, tag="fb2")
        nc.sync.dma_start(out=fb2_sb[:], in_=fb2_d[:, :])
        fw3_sb = cpool.tile([128, 2 * NCLASS], dt.bfloat16, tag="fw3")
        nc.sync.dma_start(out=fw3_sb[:], in_=fw3_d[:, :])
        fb3_sb = cpool.tile([NCLASS, 1], dt.float32, tag="fb3")
        nc.sync.dma_start(out=fb3_sb[:], in_=fb3_d[:, :])
        identp_sb = cpool.tile([NFEAT, ROWPAD], dt.bfloat16, tag="identp")
        nc.sync.dma_start(out=identp_sb[:], in_=identp_d[:, :])
        zero64_sb = cpool.tile([128, NFEAT], dt.bfloat16, tag="zero64")
        nc.gpsimd.memset(zero64_sb[:], 0.0)
        cmax_sb = small_pool.tile([128, 8], dt.float32, tag="cmax")

        ntile_tr = math.ceil(shard / 128)

        for l in range(4):
            nf = feats[l]
            xT_full = xT_pool.tile([128, shard], dt.bfloat16, tag="xT")
            xT_sb = xT_full[:nf, :]
            for s in range(nsup):
                s0, s1 = st.sup_bounds[s]
                sw = s1 - s0
                pagg = ps_agg.tile([64, SUPER], dt.float32, tag="pagg")
                first_mm = True
                for t in range(NQ):
                    colspans = st.plan[(s, t)]
                    ncols = len(colspans)
                    nidx = ncols * 128
                    co = st.col_off[(s, t)]
                    so = st.s_off[(s, t)]
                    # stream idx + S for this (s, t)
                    ix = ix_pool.tile([128, max_cols * 8], dt.int16, tag="ix")
                    nc.sync.dma_start(
                        out=ix[:, : nidx // 16],
                        in_=idx_d[:, co * 8 : co * 8 + nidx // 16],
                    )
                    swst = st.sw[(s, t)]
                    sv = sv_pool.tile([128, max_sw], dt.bfloat16, tag="sv")
                    nc.sync.dma_start(
                        out=sv[:, :swst], in_=sv_d[:, so : so + swst]
                    )
                    msg = msg_pool.tile(
                        [128, max_cols * NFEAT], dt.bfloat16, tag="msg"
                    )
                    _dma_gather_raw(
                        nc.gpsimd,
                        msg[:, : ncols * NFEAT].rearrange(
                            "p (c e) -> p c e", e=NFEAT
                        ),
                        quarter_ap(l, t),
                        ix[:, : nidx // 16],
                        nidx,
                        NFEAT,
                        ROWPAD,
                        queue_num=(s + t) % 4,
                    )
                    if first_mm:
                        # full-width zeroing matmul: start=True clears the
                        # bank and gives every data matmul a WAW dep on it
                        # (data matmuls then accumulate in any order).
                        nc.tensor.matmul(
                            out=pagg[:, :sw],
                            lhsT=zero64_sb[:, :],
                            rhs=sv[:, :sw],
                            start=True, stop=False,
                            skip_group_check=True,
                        )
                        first_mm = False
                    woff = 0
                    for k, (a, b) in enumerate(colspans):
                        nc.tensor.matmul(
                            out=pagg[:, a - s0 : b - s0],
                            lhsT=msg[:, k * NFEAT : (k + 1) * NFEAT],
                            rhs=sv[:, woff : woff + (b - a)],
                            start=False,
                            stop=(t == NQ - 1 and k == len(colspans) - 1),
                            skip_group_check=True,
                        )
                        woff += b - a
                agg_sb = agg_pool.tile([64, SUPER], dt.bfloat16, tag="aggsb")
                nc.vector.tensor_copy(out=agg_sb[:, :sw], in_=pagg[:, :sw])
                pxt = ps_x.tile([128, SUPER], dt.float32, tag="px")
                px = pxt[:nf, :]
                nc.tensor.matmul(
                    out=px[:, :sw],
                    lhsT=wl_sb[l][:, :],
                    rhs=agg_sb[:, :sw],
                    start=True,
                    stop=True,
                )
                nc.scalar.activation(
                    out=xT_sb[:, s0:s1],
                    in_=px[:, :sw],
                    func=AF.Relu,
                    bias=bl_sb[l][:, :],
                    scale=1.0,
                )
                if l < 3:
                    # node-major (row-padded) transpose of this supertile
                    # via TensorE, staged to hA/hB for the chunked AG
                    t0i = (s0 // 128)
                    t1i = math.ceil(s1 / 128)
                    nt_s = t1i - t0i
                    xnode = xnode_pool.tile(
                        [128, 4 * ROWPAD], dt.bfloat16, tag="xnode"
                    )
                    for j in range(nt_s):
                        n0 = (t0i + j) * 128
                        n1 = min(n0 + 128, shard)
                        pt = ps_t.tile([128, ROWPAD], dt.float32, tag="pt")
                        nc.tensor.matmul(
                            out=pt[: n1 - n0, :],
                            lhsT=xT_sb[:, n0:n1],
                            rhs=identp_sb[:, :],
                            start=True,
                            stop=True,
                        )
                        nc.vector.tensor_copy(
                            out=xnode[: n1 - n0, j * ROWPAD : (j + 1) * ROWPAD],
                            in_=pt[: n1 - n0, :],
                        )
                    rows = s1 - s0
                    nfull = rows // 128
                    rem = rows % 128
                    if s < SUP_A:
                        dst, off = hA_d[l + 1], s0
                    elif s < SUP_B:
                        dst, off = hB_d[l + 1], s0 - HALF_A
                    else:
                        dst, off = hC_d[l + 1], s0 - HALF_A - HALF_B
                    if nfull:
                        nc.sync.dma_start(
                            out=dst[off : off + nfull * 128, :].rearrange(
                                "(t p) f -> p t f", p=128
                            ),
                            in_=xnode[:, : nfull * ROWPAD].rearrange(
                                "p (t f) -> p t f", f=ROWPAD
                            ),
                        )
                    if rem:
                        nc.sync.dma_start(
                            out=dst[off + nfull * 128 : off + rows, :],
                            in_=xnode[:rem, nfull * ROWPAD : (nfull + 1) * ROWPAD],
                        )
                if l < 3 and s == SUP_A + 1 and not profile:
                    # chunk-A AllGather: overlaps remaining supertiles
                    nc.gpsimd.collective_compute(
                        "AllGather", ALU.bypass, replica_groups=rg,
                        ins=[hA_d[l + 1][:, :]], outs=[tabA_d[l + 1][:, :]],
                    )
                if l < 3 and s == SUP_B + 1 and not profile:
                    nc.gpsimd.collective_compute(
                        "AllGather", ALU.bypass, replica_groups=rg,
                        ins=[hB_d[l + 1][:, :]], outs=[tabB_d[l + 1][:, :]],
                    )
                if l == 3:
                    # fused 320->1024 combination + max-pool for this chunk
                    q0 = comb_pool.tile([128, SUPER], dt.bfloat16, tag="q0")
                    nc.sync.dma_start(out=q0[0:64, :sw], in_=xiT_d[0][:, s0:s1])
                    nc.sync.dma_start(out=q0[64:128, :sw], in_=xiT_d[1][:, s0:s1])
                    q1 = comb_pool.tile([64, SUPER], dt.bfloat16, tag="q1")
                    nc.sync.dma_start(out=q1[:, :sw], in_=xiT_d[2][:, s0:s1])
                    for m in range(8):
                        pc = ps_c.tile([128, SUPER], dt.float32, tag="pc")
                        nc.tensor.matmul(
                            out=pc[:, :sw],
                            lhsT=wcb_sb[0:128, m * 128 : (m + 1) * 128],
                            rhs=q0[:, :sw],
                            start=True, stop=False,
                        )
                        nc.tensor.matmul(
                            out=pc[:, :sw],
                            lhsT=wcb_sb[0:64, 1024 + m * 128 : 1024 + (m + 1) * 128],
                            rhs=q1[:, :sw],
                            start=False, stop=False,
                        )
                        nc.tensor.matmul(
                            out=pc[:, :sw],
                            lhsT=wcb_sb[0:128, 2048 + m * 128 : 2048 + (m + 1) * 128],
                            rhs=xT_full[:, s0:s1],
                            start=False, stop=True,
                        )
                        red = small_pool.tile([128, 1], dt.float32, tag="red")
                        nc.vector.tensor_reduce(
                            out=red[:, :],
                            in_=pc[:, :sw],
                            axis=mybir.AxisListType.X,
                            op=ALU.max,
                        )
                        if s == 0:
                            nc.vector.tensor_copy(
                                out=cmax_sb[:, m : m + 1], in_=red[:, :]
                            )
                        else:
                            nc.vector.tensor_tensor(
                                out=cmax_sb[:, m : m + 1],
                                in0=cmax_sb[:, m : m + 1],
                                in1=red[:, :],
                                op=ALU.max,
                            )
            if l < 3:
                if not profile:
                    nc.gpsimd.collective_compute(
                        "AllGather", ALU.bypass, replica_groups=rg,
                        ins=[hC_d[l + 1][:, :]], outs=[tabC_d[l + 1][:, :]],
                    )
                # spill xT for the combination stage
                nc.sync.dma_start(out=xiT_d[l][:, :], in_=xT_sb[:, :])

        # ---------------- all-reduce max + head ----------------
        conv_stk.close()
        ps_h = stk.enter_context(tc.tile_pool(name="ps_h", bufs=2, space="PSUM"))
        if profile:
            gmax_sb = cmax_sb
        else:
            nc.sync.dma_start(out=cmax_d[:, :], in_=cmax_sb[:, :])
            nc.gpsimd.collective_compute(
                "AllReduce", ALU.max, replica_groups=rg,
                ins=[cmax_d[:, :]], outs=[gmax_d[:, :]],
            )
            gmax_sb = small_pool.tile([128, 8], dt.float32, tag="gmax")
            nc.sync.dma_start(out=gmax_sb[:, :], in_=gmax_d[:, :])
        h0 = small_pool.tile([128, 8], dt.bfloat16, tag="h0")
        hpre = small_pool.tile([128, 8], dt.float32, tag="hpre")
        nc.vector.tensor_tensor(
            out=hpre[:, :], in0=gmax_sb[:, :], in1=bcb_sb[:, :], op=ALU.add
        )
        nc.vector.tensor_relu(out=h0[:, :], in_=hpre[:, :])

        h1 = small_pool.tile([128, 4], dt.bfloat16, tag="h1")
        for o in range(4):
            ph = ps_h.tile([128, 1], dt.float32, tag="ph")
            for q in range(8):
                nc.tensor.matmul(
                    out=ph[:, :],
                    lhsT=fw1_sb[:, q * 512 + o * 128 : q * 512 + (o + 1) * 128],
                    rhs=h0[:, q : q + 1],
                    start=(q == 0), stop=(q == 7),
                )
            nc.scalar.activation(
                out=h1[:, o : o + 1], in_=ph[:, :], func=AF.Relu,
                bias=fb1_sb[:, o : o + 1], scale=1.0,
            )
        h2 = small_pool.tile([128, 2], dt.bfloat16, tag="h2")
        for o in range(2):
            ph = ps_h.tile([128, 1], dt.float32, tag="ph")
            for q in range(4):
                nc.tensor.matmul(
                    out=ph[:, :],
                    lhsT=fw2_sb[:, q * 256 + o * 128 : q * 256 + (o + 1) * 128],
                    rhs=h1[:, q : q + 1],
                    start=(q == 0), stop=(q == 3),
                )
            nc.scalar.activation(
                out=h2[:, o : o + 1], in_=ph[:, :], func=AF.Relu,
                bias=fb2_sb[:, o : o + 1], scale=1.0,
            )
        po = ps_h.tile([NCLASS, 1], dt.float32, tag="po")
        for q in range(2):
            nc.tensor.matmul(
                out=po[:, :],
                lhsT=fw3_sb[:, q * NCLASS : (q + 1) * NCLASS],
                rhs=h2[:, q : q + 1],
                start=(q == 0), stop=(q == 1),
            )
        out_sb = small_pool.tile([NCLASS, 1], dt.float32, tag="outsb")
        nc.vector.tensor_tensor(
            out=out_sb[:, :], in0=po[:, :], in1=fb3_sb[:, :], op=ALU.add
        )
        nc.sync.dma_start(
            out=out_d.ap().rearrange("a b -> b a"), in_=out_sb[:, :]
        )

    nc.compile()
    return nc


# ----------------------------------------------------------------------
# Host wrapper
# ----------------------------------------------------------------------

def make_inputs(inputs, st, idx_all, sv_all, n_nodes, n_cores):
    shard = n_nodes // n_cores
    x = _f32(inputs["x"])
    s_bn = lambda g: _f32(g) / np.sqrt(np.float32(1.0) + np.float32(BN_EPS))

    def pack_lhsT(w, kdim, mdim):  # w: [out, in] -> [128, (kdim/128)*mdim]
        nq = kdim // 128
        arr = np.zeros((128, nq * mdim), np.float32)
        for q in range(nq):
            arr[:, q * mdim : (q + 1) * mdim] = w[:, q * 128 : (q + 1) * 128].T
        return _bf16(arr)

    s1, s2 = s_bn(inputs["g1"]), s_bn(inputs["g2"])
    w1 = _f32(inputs["fc1_w"]) * s1[:, None]
    b1 = _f32(inputs["fc1_b"]) * s1 + _f32(inputs["be1"])
    w2 = _f32(inputs["fc2_w"]) * s2[:, None]
    b2 = _f32(inputs["fc2_b"]) * s2 + _f32(inputs["be2"])
    w3 = _f32(inputs["fc3_w"])
    b3 = _f32(inputs["fc3_b"])

    fw1 = pack_lhsT(w1, 1024, 512)
    fb1 = np.zeros((128, 4), np.float32)
    for o in range(4):
        fb1[:, o] = b1[o * 128 : (o + 1) * 128]
    fw2 = pack_lhsT(w2, 512, 256)
    fb2 = np.zeros((128, 2), np.float32)
    for o in range(2):
        fb2[:, o] = b2[o * 128 : (o + 1) * 128]
    fw3 = pack_lhsT(w3, 256, NCLASS)
    fb3 = b3[:, None]

    wcb = _f32(inputs["Wcb"])  # [320, 1024]
    wcb_p = np.zeros((128, 3 * 1024), np.float32)
    wcb_p[0:128, 0:1024] = wcb[0:128]       # vs q0 = [x1; x2]
    wcb_p[0:64, 1024:2048] = wcb[128:192]   # vs q1 = [x3]
    wcb_p[0:128, 2048:3072] = wcb[192:320]  # vs x4 (straight from SBUF)
    bcb = _f32(inputs["bcb"])
    bcb_p = np.zeros((128, 8), np.float32)
    for m in range(8):
        bcb_p[:, m] = bcb[m * 128 : (m + 1) * 128]

    identp = np.zeros((NFEAT, ROWPAD), np.float32)
    identp[:, :NFEAT] = np.eye(NFEAT)

    # full permuted layer-0 table, replicated on every core
    nodes = np.arange(n_nodes)
    tq, tidx = _rowmap(nodes)
    xa = np.zeros((TAB_A, ROWPAD), np.float32)
    xb = np.zeros((TAB_B, ROWPAD), np.float32)
    xc = np.zeros((TAB_C, ROWPAD), np.float32)
    ma, mb, mc = tq < 2, tq == 2, tq == 3
    xa[(tq[ma] * QSZ_A + tidx[ma]), :NFEAT] = x[nodes[ma]]
    xb[tidx[mb], :NFEAT] = x[nodes[mb]]
    xc[tidx[mc], :NFEAT] = x[nodes[mc]]

    common = dict(
        wcb=_bf16(wcb_p),
        bcb=bcb_p,
        fw1=fw1, fb1=fb1, fw2=fw2, fb2=fb2, fw3=fw3, fb3=fb3,
        identp=_bf16(identp),
        table0a=_bf16(xa), table0b=_bf16(xb), table0c=_bf16(xc),
    )
    for l in range(4):
        common[f"wl{l}"] = _bf16(inputs[f"W{l + 1}"])
        common[f"bl{l}"] = _f32(inputs[f"b{l + 1}"])[:, None]

    in_maps = []
    for c in range(n_cores):
        m = dict(common)
        m["sv"] = sv_all[c]
        m["idx"] = idx_all[c]
        in_maps.append(m)
    return in_maps


LAST_EXEC_NS = None


def _install_ntff_hook():
    """Best-effort shim for antenv.axon_hooks (absent on some images) so
    run_bass_kernel_spmd(trace=True) can capture NTFF profiles."""
    import sys
    import types
    try:
        from antenv.axon_hooks import get_axon_ntff_profile_hook  # noqa: F401
        return
    except ImportError:
        pass
    try:
        import antenv
        from trn_agent_boot.trn_boot import _ntff_profile_via_ctypes
        mod = types.ModuleType("antenv.axon_hooks")
        _hook = [None]
        mod.set_axon_ntff_profile_hook = lambda h: _hook.__setitem__(0, h)
        mod.get_axon_ntff_profile_hook = lambda: _hook[0]
        antenv.axon_hooks = mod
        sys.modules["antenv.axon_hooks"] = mod
        mod.set_axon_ntff_profile_hook(
            _ntff_profile_via_ctypes("/opt/axon/libaxon_pjrt.so")
        )
    except Exception:
        pass


def kernel(**inputs):
    global LAST_EXEC_NS
    import os
    import time

    edge_src = np.asarray(inputs["edge_src"])
    edge_dst = np.asarray(inputs["edge_dst"])
    edge_w = _f32(inputs["edge_w"])

    t0 = time.time()
    st, idx_all, sv_all = build_structure(
        edge_src, edge_dst, edge_w, N_NODES, N_CORES
    )
    t1 = time.time()
    nc = build_program(st, N_NODES, N_CORES)
    t2 = time.time()
    in_maps = make_inputs(inputs, st, idx_all, sv_all, N_NODES, N_CORES)
    trace = bool(int(os.environ.get("KERNEL_TRACE", "0")))
    if trace:
        _install_ntff_hook()
    res = run_bass_kernel_spmd(
        nc, in_maps, core_ids=list(range(N_CORES)), trace=trace
    )
    t3 = time.time()
    LAST_EXEC_NS = res.exec_time_ns
    print(
        f"[kernel] fill={st.fill:.3f} ncol={st.ncol} "
        f"prep={t1 - t0:.1f}s build+compile={t2 - t1:.1f}s "
        f"run={t3 - t2:.1f}s exec_ns={res.exec_time_ns}"
    )
    return np.asarray(res.results[0]["out"], dtype=np.float32)


if __name__ == "__main__":
    data = dict(np.load("/root/problem/inputs.npz"))
    out = kernel(**data)
    print(out)
